# revision 1
# baseline (speedup 1.0000x reference)
"""Self-contained Trainium2 Bass kernel for nn_ActorMpnn (8-core SPMD MPNN).

kernel(**inputs) takes the FULL unsharded inputs (as produced by
setup_inputs()) and returns the FULL [B, N, 8] float32 output.

Strategy: 2 cores per graph (B=4, 8 cores). Host-side preprocessing sorts
each graph's nodes by in-degree bucket, splits them into two balanced
half-shards, and lays the edges out in 512-slot PSUM tiles so the min-
aggregation becomes regular strided reductions. On-device per layer:
node-level projections (p_src table via PE transpose; r = Wdst@x + b1),
then per 4096-edge chunk: dma_gather of source-node rows (bf16, matmul-
ready layout), edge-feature rank-1 term via gpsimd partition_broadcast +
fused DVE scalar_tensor_tensor, per-tile r broadcast-add, leaky-relu on
ACT, 128x128 w2 matmul into PSUM, Lrelu(m+b2) drain (leaky/bias commute
with min), and per-tile reduce-min into the node columns. Halves are
exchanged with a 2-core AllGather through HBM. Final linear + softplus
(exp/ln) on device; host inverse-permutes the output.
"""
import sys as _sys
for _p in ('/opt/trn_rl_repo',):
    if _p not in _sys.path:
        _sys.path.insert(0, _p)
import numpy as np
import ml_dtypes


bf16 = ml_dtypes.bfloat16

K_LIST = [4, 6, 8, 10, 12, 14, 16, 18, 20, 22, 24, 28, 32, 40, 48, 64, 96, 128]
M_K = {k: 512 // k for k in K_LIST}
TILE = 512
CHUNK_TILES = 8          # gather chunk = 8 tiles = 4096 edges
B, N, E = 4, 8192, 131072
NODE_F, HID = 16, 128


def bucket_of(d):
    for k in K_LIST:
        if k >= d:
            return k
    raise AssertionError(f"degree {d} exceeds max bucket")


def build_layout(edge_index):
    """edge_index: [B, 2, E] int array. Returns layout dict."""
    per_graph = []
    for g in range(B):
        src = np.asarray(edge_index[g, 0], dtype=np.int64)
        dst = np.asarray(edge_index[g, 1], dtype=np.int64)
        deg = np.bincount(dst, minlength=N)
        assert deg.max() <= K_LIST[-1]
        buck = np.array([bucket_of(d) if d > 0 else 0 for d in deg])
        # nodes with deg>0 sorted by (bucket, node id); alternate nodes of
        # each bucket between the two halves so bucket counts stay balanced
        active = np.nonzero(deg > 0)[0]
        order = active[np.lexsort((active, buck[active]))]
        halves = [order[0::2], order[1::2]]
        zeros = np.nonzero(deg == 0)[0]
        # deg-0 nodes split evenly (they only need table slots + final output)
        zhalves = [zeros[: len(zeros) // 2], zeros[len(zeros) // 2:]]
        # CSR by dst
        eorder = np.argsort(dst, kind="stable")
        starts = np.zeros(N + 1, np.int64)
        starts[1:] = np.cumsum(deg)
        per_graph.append(dict(src=src, dst=dst, deg=deg, buck=buck,
                              halves=halves, zhalves=zhalves,
                              eorder=eorder, starts=starts))

    # uniform bucket counts across the 8 shards
    counts = {k: 0 for k in K_LIST}
    for pg in per_graph:
        for h in range(2):
            b = pg["buck"][pg["halves"][h]]
            for k in K_LIST:
                counts[k] = max(counts[k], int((b == k).sum()))
    # round node counts up to full tiles
    N_k = {}
    for k in K_LIST:
        m = M_K[k]
        N_k[k] = int(np.ceil(counts[k] / m) * m) if counts[k] else 0
    n_zero = max(max(len(z) for pg in per_graph for z in pg["zhalves"]), 0)

    C_active = sum(N_k.values())
    C = C_active + n_zero
    C = int(np.ceil(C / 128) * 128)          # table-slot alignment
    T = sum(N_k[k] // M_K[k] for k in K_LIST)
    T = int(np.ceil(T / 16) * 16)            # ea block-wrap needs T % 16 == 0
    E_pad = T * TILE

    # per-tile schedule (same for every core): list of (k, m_k, out_base)
    tiles = []
    out_base = 0
    for k in K_LIST:
        m = M_K[k]
        for t in range(N_k[k] // m if N_k[k] else 0):
            tiles.append((k, m, out_base))
            out_base += m
    while len(tiles) < T:                    # filler tiles (never reduced)
        tiles.append((0, 0, out_base))

    layout = dict(N_k=N_k, C=C, C_active=C_active, T=T, E_pad=E_pad,
                  n_zero=n_zero, tiles=tiles, per_graph=per_graph)

    # per-core shard data
    shards = []
    for g in range(B):
        pg = per_graph[g]
        # global slot map for this graph: slot = core_half*C + position
        slotof = np.full(N, -1, np.int64)
        core_nodes = []  # per half: list of (slot_pos -> node or -1 dummy)
        for h in range(2):
            nodes_h = pg["halves"][h]
            b_h = pg["buck"][nodes_h]
            pos_nodes = np.full(C, -1, np.int64)
            p = 0
            for k in K_LIST:
                sel = nodes_h[b_h == k]
                pos_nodes[p:p + len(sel)] = sel
                p += N_k[k]
            zh = pg["zhalves"][h]
            pos_nodes[p:p + len(zh)] = zh
            core_nodes.append(pos_nodes)
            real = pos_nodes >= 0
            slotof[pos_nodes[real]] = h * C + np.nonzero(real)[0]
        assert (slotof[pg["deg"] > 0] >= 0).all()

        for h in range(2):
            pos_nodes = core_nodes[h]
            # edge slots
            idx = np.zeros(E_pad, np.int64)      # gather table slot per edge slot
            eav = np.zeros(E_pad, np.float32)    # ea value per edge slot
            # iterate tiles: per tile (k, m, out_base): m segments
            node_ptr = {k: 0 for k in K_LIST}    # node position within bucket
            # bucket base position (in node-slot space) per k
            bases = {}
            p = 0
            for k in K_LIST:
                bases[k] = p
                p += N_k[k]
            for t, (k, m, ob) in enumerate(layout["tiles"]):
                es = t * TILE
                if k == 0:
                    # filler tile: point at slot of core-half base (any valid)
                    idx[es:es + TILE] = h * C
                    continue
                for j in range(m):
                    pos = ob + j
                    node = pos_nodes[pos]
                    if node < 0:
                        # dummy node: k dup edges pointing at table slot h*C
                        idx[es + j * k: es + (j + 1) * k] = h * C
                        eav[es + j * k: es + (j + 1) * k] = 0.0
                    else:
                        d = pg["deg"][node]
                        e0 = pg["starts"][node]
                        eids = pg["eorder"][e0:e0 + d]
                        ss = slotof[pg["src"][eids]]
                        assert (ss >= 0).all()
                        reps = np.concatenate([ss, np.repeat(ss[:1], k - d)])
                        erep = np.concatenate([eids, np.repeat(eids[:1], k - d)])
                        idx[es + j * k: es + (j + 1) * k] = reps
                        eav[es + j * k: es + (j + 1) * k] = 0.0  # set below
                        eav[es + j * k: es + (j + 1) * k] = -1  # placeholder
                        idx_slice = slice(es + j * k, es + (j + 1) * k)
                        eav[idx_slice] = 0.0
                        eav[idx_slice] = np.asarray(
                            _EA_CACHE[g][erep], dtype=np.float32)
                # tail slots of the tile: dup the last real edge slot
                tail0 = es + m * k
                if tail0 < es + TILE:
                    idx[tail0: es + TILE] = idx[tail0 - 1]
                    eav[tail0: es + TILE] = eav[tail0 - 1]

            shards.append(dict(g=g, h=h, idx=idx, ea=eav,
                               pos_nodes=pos_nodes, slotof=slotof))
    layout["shards"] = shards
    return layout


_EA_CACHE = None


def preprocess(node_features, edge_index, edge_features):
    """Build all per-core input arrays. Returns (layout, in_maps_extra)."""
    global _EA_CACHE
    _EA_CACHE = [np.asarray(edge_features[g, :, 0], np.float32) for g in range(B)]
    layout = build_layout(edge_index)
    C, T, E_pad = layout["C"], layout["T"], layout["E_pad"]
    per_core = []
    for ci, sh in enumerate(layout["shards"]):
        g, h = sh["g"], sh["h"]
        # wrapped gather idx: [16, E_pad/16] int16, i -> (i%16, i//16)
        idx = sh["idx"].astype(np.int16)
        idx_w = idx.reshape(E_pad // 16, 16).T.copy()
        idx_w = np.vstack([idx_w, idx_w]).copy()   # partitions 0:16 (sim) + 16:32 (hw)
        # ea block-wrap: [16, E_pad/16] partition p holds slots [p*L, (p+1)*L)
        L = E_pad // 16
        ea_bw = sh["ea"].reshape(16, L).astype(bf16)
        # x0 columns for own nodes [16, C] f32 (0 for dummy slots)
        x0 = np.asarray(node_features[g], np.float32)   # [N, 16]
        x0_own = np.zeros((NODE_F, C), np.float32)
        pos = sh["pos_nodes"]
        real = pos >= 0
        x0_own[:, np.nonzero(real)[0]] = x0[pos[real]].T
        # x0 full (both halves) [16, 2C]
        other = layout["shards"][ci ^ 1]
        x0_full = np.zeros((NODE_F, 2 * C), np.float32)
        x0_full[:, h * C:(h + 1) * C] = x0_own[:, :C]
        opos = other["pos_nodes"]
        oreal = opos >= 0
        x0_full[:, (1 - h) * C + np.nonzero(oreal)[0]] = x0[opos[oreal]].T
        per_core.append(dict(idx_w=idx_w, ea_bw=ea_bw,
                             x0_own=x0_own.astype(bf16),
                             x0_full=x0_full.astype(bf16)))
    return layout, per_core


def postprocess(layout, outs):
    """outs: list of 8 arrays [8, C] f32 (per-core final). Returns [B, N, 8]."""
    C = layout["C"]
    res = np.zeros((B, N, 8), np.float32)
    for ci, sh in enumerate(layout["shards"]):
        g = sh["g"]
        pos = sh["pos_nodes"]
        real = pos >= 0
        res[g, pos[real]] = outs[ci][:, np.nonzero(real)[0]].T
    return res


# ======================= device kernel builder =======================
from contextlib import ExitStack
import concourse.bass as bass
import concourse.tile as tile
from concourse import bacc, mybir
from concourse.bass_utils import run_bass_kernel_spmd


bf16 = ml_dtypes.bfloat16
FP = mybir.dt.float32
BF = mybir.dt.bfloat16
AF = mybir.ActivationFunctionType
OP = mybir.AluOpType

HID = 128
NODE_F = 16
CHUNK = 4096          # edges per gather chunk (8 tiles)
SIM_COMPAT = False    #替换 Lrelu with DVE ops for CoreSim
CH_TILES = 8


def build_kernel(C, T, tiles, reps=1, debug=False, stage="full", max_chunks=10**9):
    # stage: node < gather < asm < mm < nocc < full
    SLVL = ["node", "gather", "asm", "mm", "nocc", "full"].index(stage)
    """tiles: list of (k, m, out_base) per 512-slot tile."""
    E_pad = T * 512
    n_chunks = T // CH_TILES
    nc = bacc.Bacc(num_devices=8)

    # ---------------- DRAM I/O ----------------
    D = {}
    def din(name, shape, dt):
        D[name] = nc.dram_tensor(name, shape, dt, kind="ExternalInput")
    din("idx", [32, E_pad // 16], mybir.dt.int16)
    din("ea", [1, E_pad], BF)
    din("x0o", [NODE_F, C], BF)
    din("x0f", [NODE_F, 2 * C], BF)
    for l in range(3):
        fin = NODE_F if l == 0 else HID
        din(f"wsrc{l}", [fin, HID], BF)
        din(f"wdst{l}", [fin, HID], BF)
        din(f"weac{l}", [HID, 1], FP)
        din(f"b1c{l}", [HID, 1], FP)
        din(f"w2{l}", [HID, HID], BF)
        din(f"b2c{l}", [HID, 1], FP)
    din("lw16", [NODE_F, 8], BF)
    din("lw128", [HID, 8], BF)
    din("lbc", [8, 1], FP)

    out_d = nc.dram_tensor("out", [8, C], FP, kind="ExternalOutput")
    dbg = {}
    if debug:
        for l in range(3):
            dbg[l] = nc.dram_tensor(f"dbgx{l}", [HID, C], FP, kind="ExternalOutput")

    ag_in = [nc.dram_tensor(f"agin{l}", [HID, C], BF) for l in range(2)]
    ag_out = [nc.dram_tensor(f"agout{l}", [2, HID, C], BF) for l in range(2)]

    ident_dram = nc.inline_tensor(np.eye(128, dtype=np.float32).astype(bf16),
                                  name="ident_c")

    with tile.TileContext(nc, num_cores=8) as tc:
      with ExitStack() as ctx:
        # ---------------- pools ----------------
        persist = ctx.enter_context(tc.tile_pool(name="persist", bufs=1))
        chp = ctx.enter_context(tc.tile_pool(name="chunks", bufs=2))
        sdp = ctx.enter_context(tc.tile_pool(name="sdrain", bufs=4))
        pp_m = ctx.enter_context(tc.tile_pool(name="ppm", bufs=4, space="PSUM"))
        pp_proj = ctx.enter_context(tc.tile_pool(name="ppp", bufs=2, space="PSUM"))
        pp_tp = ctx.enter_context(tc.tile_pool(name="ppt", bufs=1, space="PSUM"))
        pp_fin = ctx.enter_context(tc.tile_pool(name="ppf", bufs=1, space="PSUM"))

        # ---------------- persistent tiles ----------------
        idx_t = persist.tile([128, E_pad // 16], mybir.dt.int16)
        nc.vector.memset(idx_t[:], 0)
        nc.sync.dma_start(idx_t[0:32, :], D["idx"].ap())

        x0o_t = persist.tile([NODE_F, C], BF)
        nc.sync.dma_start(x0o_t[:], D["x0o"].ap())
        x0f_t = persist.tile([NODE_F, 2 * C], BF)
        nc.sync.dma_start(x0f_t[:], D["x0f"].ap())

        W = {}
        for l in range(3):
            fin = NODE_F if l == 0 else HID
            for nm, sh, dt in [(f"wsrc{l}", [fin, HID], BF),
                               (f"wdst{l}", [fin, HID], BF),
                               (f"weac{l}", [HID, 1], FP),
                               (f"b1c{l}", [HID, 1], FP),
                               (f"w2{l}", [HID, HID], BF),
                               (f"b2c{l}", [HID, 1], FP)]:
                W[nm] = persist.tile(sh, dt, name=nm, tag=nm)
                nc.sync.dma_start(W[nm][:], D[nm].ap())
        for nm, sh, dt in [("lw16", [NODE_F, 8], BF), ("lw128", [HID, 8], BF),
                           ("lbc", [8, 1], FP)]:
            W[nm] = persist.tile(sh, dt, name=nm, tag=nm)
            nc.sync.dma_start(W[nm][:], D[nm].ap())
        ident = persist.tile([128, 128], BF)
        nc.sync.dma_start(ident[:], ident_dram.ap())

        n_stripe = 2 * C // 128
        table = persist.tile([128, n_stripe * 128], BF)   # row-layout gather table
        x_own = persist.tile([HID, C], BF)
        x_full = persist.tile([HID, 2 * C], BF)
        r_cols = persist.tile([HID, C], BF)
        x3_own = persist.tile([HID, C], BF)
        pcols = persist.tile([128, 2 * C], BF)

        def node_proj(lhsT, rhs_t, ncols, drain_bias, out_cols):
            """out_cols[:, :ncols] (bf16) = lhsT.T @ rhs_t[:, :ncols] (+bias)"""
            for j0 in range(0, ncols, 512):
                w5 = min(512, ncols - j0)
                pb = pp_proj.tile([128, 512], FP, tag="proj")
                nc.tensor.matmul(pb[:, :w5], lhsT, rhs_t[:, j0:j0 + w5],
                                 start=True, stop=True)
                if drain_bias is not None:
                    nc.scalar.activation(out_cols[:, j0:j0 + w5], pb[:, :w5],
                                         AF.Identity, bias=drain_bias, scale=1.0)
                else:
                    nc.vector.tensor_copy(out_cols[:, j0:j0 + w5], pb[:, :w5])

        def build_table(src_cols, ncols):
            """table rows <- transpose of src_cols [128, ncols] (bf16)."""
            for j0 in range(0, ncols, 128):
                pt = pp_tp.tile([128, 128], BF, tag="tp")
                nc.tensor.transpose(pt[:], src_cols[:, j0:j0 + 128], ident[:])
                nc.vector.tensor_copy(table[:, j0:j0 + 128], pt[:])

        for rep in range(reps):
            for l in range(3):
                # ---- node phase ----
                if l == 0:
                    xf, xo = x0f_t, x0o_t
                else:
                    xf, xo = x_full, x_own
                node_proj(W[f"wsrc{l}"][:], xf[:], 2 * C, None, pcols)
                build_table(pcols, 2 * C)
                node_proj(W[f"wdst{l}"][:], xo[:], C, W[f"b1c{l}"][:], r_cols)

                x_dst = x3_own if l == 2 else x_own
                nc.vector.memset(x_dst[:], 0.0)

                # ---- edge phase ----
                for ch in range(n_chunks):
                    if SLVL < 1 or ch >= max_chunks:
                        break
                    ch_tiles = tiles[ch * CH_TILES:(ch + 1) * CH_TILES]
                    if all(k == 0 for (k, m, ob) in ch_tiles):
                        continue
                    e0 = ch * CHUNK
                    # gather (512-idx pieces: single_packet works up to 512)
                    G = chp.tile([128, CHUNK], BF, tag="G")
                    for gj in range(CHUNK // 512):
                        i0 = e0 // 16 + gj * 32
                        nc.gpsimd.dma_gather(
                            out_ap=G[:, gj * 512:(gj + 1) * 512].rearrange(
                                "p (o n) -> p o n", o=1),
                            in_ap=table[:],
                            idxs_ap=idx_t[:, i0:i0 + 32],
                            num_idxs=512, num_idxs_reg=512, elem_size=128,
                            transpose=True, sbuf_tokens_per_rank=128,
                            sbuf_free_dim_per_rank=256, single_packet=True)
                    if SLVL < 2:
                        continue
                    # ea broadcast
                    ea1 = chp.tile([1, CHUNK], BF, tag="ea1")
                    nc.sync.dma_start(ea1[:], D["ea"].ap()[:, e0:e0 + CHUNK])
                    eab = chp.tile([128, CHUNK], BF, tag="eab")
                    nc.gpsimd.partition_broadcast(eab[:], ea1[:])
                    # t = eab*weac + G   (frees G for the next gather)
                    tband = chp.tile([128, CHUNK], BF, tag="tband")
                    nc.vector.scalar_tensor_tensor(
                        tband[:], eab[:], W[f"weac{l}"][:], G[:],
                        op0=OP.mult, op1=OP.add)
                    G = tband
                    # per-tile r add (in place)
                    for j, (k, m, ob) in enumerate(ch_tiles):
                        if k == 0:
                            continue
                        sl = slice(j * 512, j * 512 + m * k)
                        r_sl = r_cols[:, ob:ob + m]
                        r_bc = bass.AP(tensor=r_sl.tensor, offset=r_sl.offset,
                                       ap=[list(r_sl.ap[0]), list(r_sl.ap[1]),
                                           [0, k]])
                        gv = G[:, sl].rearrange("p (n k) -> p n k", k=k)
                        nc.vector.tensor_tensor(gv, gv, r_bc, op=OP.add)
                    # leaky1 chunk-wide (in place)
                    if SIM_COMPAT:
                        nc.vector.scalar_tensor_tensor(
                            G[:], G[:], 0.01, G[:], op0=OP.mult, op1=OP.max)
                    else:
                        nc.scalar.activation(G[:], G[:], AF.Lrelu,
                                             bias=0.0, scale=1.0, alpha=0.01)
                    if SLVL < 3:
                        continue
                    # per-tile: w2 matmul, drain(+b2,leaky2), reduce-min
                    for j, (k, m, ob) in enumerate(ch_tiles):
                        if k == 0:
                            continue
                        mb = pp_m.tile([128, 512], FP, tag="mpsum")
                        nc.tensor.matmul(mb[:], W[f"w2{l}"][:],
                                         G[:, j * 512:(j + 1) * 512],
                                         start=True, stop=True)
                        s = sdp.tile([128, 512], BF, tag="sdrain")
                        if SIM_COMPAT:
                            nc.scalar.activation(s[:, :m * k], mb[:, :m * k],
                                                 AF.Identity,
                                                 bias=W[f"b2c{l}"][:], scale=1.0)
                            nc.vector.scalar_tensor_tensor(
                                s[:, :m * k], s[:, :m * k], 0.01, s[:, :m * k],
                                op0=OP.mult, op1=OP.max)
                        else:
                            nc.scalar.activation(s[:, :m * k], mb[:, :m * k], AF.Lrelu,
                                                 bias=W[f"b2c{l}"][:], scale=1.0,
                                                 alpha=0.01)
                        nc.vector.tensor_reduce(
                            x_dst[:, ob:ob + m],
                            s[:, :m * k].rearrange("p (n k) -> p n k", k=k),
                            axis=mybir.AxisListType.X, op=OP.min)

                if debug:
                    dd = persist.tile([128, C], FP, tag="dbgd")
                    nc.vector.tensor_copy(dd[:], x_dst[:])
                    nc.sync.dma_start(dbg[l].ap(), dd[:])

                # ---- exchange (layers 0,1) ----
                if l < 2:
                    if SLVL >= 5:
                        nc.sync.dma_start(ag_in[l].ap(), x_own[:])
                        tc.strict_bb_all_engine_barrier()
                        nc.gpsimd.collective_compute(
                            "AllGather", OP.bypass,
                            replica_groups=[[0, 1], [2, 3], [4, 5], [6, 7]],
                            ins=[ag_in[l].ap()], outs=[ag_out[l].ap()])
                        tc.strict_bb_all_engine_barrier()
                        nc.sync.dma_start(x_full[:, 0:C], ag_out[l].ap()[0])
                        nc.sync.dma_start(x_full[:, C:2 * C], ag_out[l].ap()[1])
                    else:
                        nc.vector.tensor_copy(x_full[:, 0:C], x_own[:])
                        nc.vector.tensor_copy(x_full[:, C:2 * C], x_own[:])

            # ---- final linear + softplus ----
            for j0 in range(0, C, 512):
                w5 = min(512, C - j0)
                pb = pp_fin.tile([8, 512], FP, tag="fin")
                nc.tensor.matmul(pb[:, :w5], W["lw16"][:], x0o_t[:, j0:j0 + w5],
                                 start=True, stop=False)
                nc.tensor.matmul(pb[:, :w5], W["lw128"][:], x3_own[:, j0:j0 + w5],
                                 start=False, stop=True)
                ex = sdp.tile([8, 512], FP, tag="fex")
                nc.scalar.activation(ex[:, :w5], pb[:, :w5], AF.Exp,
                                     bias=W["lbc"][:], scale=1.0)
                oo = sdp.tile([8, 512], FP, tag="fo")
                nc.scalar.activation(oo[:, :w5], ex[:, :w5], AF.Ln, bias=1.0, scale=1.0)
                nc.sync.dma_start(out_d.ap()[:, j0:j0 + w5], oo[:, :w5])

    nc.finalize()
    return nc


def make_in_maps(layout, per_core, inputs):
    """Build per-core in_maps from layout + preprocess() output + raw inputs."""
    maps = []
    for ci in range(8):
        pc = per_core[ci]
        m = dict(idx=pc["idx_w"], ea=pc["ea_bw"].reshape(1, -1),
                 x0o=pc["x0_own"], x0f=pc["x0_full"])
        for l in range(3):
            fin = NODE_F if l == 0 else HID
            w1 = np.asarray(inputs[f"c{l+1}_w1"], np.float32)
            m[f"wdst{l}"] = w1[:fin].astype(bf16)
            m[f"wsrc{l}"] = w1[fin:2 * fin].astype(bf16)
            m[f"weac{l}"] = w1[2 * fin].reshape(HID, 1).astype(np.float32)
            m[f"b1c{l}"] = np.asarray(inputs[f"c{l+1}_b1"], np.float32).reshape(HID, 1)
            m[f"w2{l}"] = np.asarray(inputs[f"c{l+1}_w2"], np.float32).astype(bf16)
            m[f"b2c{l}"] = np.asarray(inputs[f"c{l+1}_b2"], np.float32).reshape(HID, 1)
        lw = np.asarray(inputs["lin_w"], np.float32)
        m["lw16"] = lw[:NODE_F].astype(bf16)
        m["lw128"] = lw[NODE_F:].astype(bf16)
        m["lbc"] = np.asarray(inputs["lin_b"], np.float32).reshape(8, 1)
        maps.append(m)
    return maps


_CACHE = {}


def kernel(node_features, edge_index, edge_features, **weights):
    inputs = dict(weights)
    layout, per_core = preprocess(node_features, edge_index, edge_features)
    key = (layout["C"], layout["T"])
    if key not in _CACHE:
        _CACHE[key] = build_kernel(layout["C"], layout["T"], layout["tiles"],
                                   reps=1, debug=False, stage="full")
    nc = _CACHE[key]
    in_maps = make_in_maps(layout, per_core, inputs)
    res = run_bass_kernel_spmd(nc, in_maps, core_ids=list(range(8)))
    outs = [np.asarray(res.results[c]["out"]) for c in range(8)]
    return postprocess(layout, outs).astype(np.float32)

